# revision 8
# baseline (speedup 1.0000x reference)
"""De-stationary causal attention (B=2, L=S=2048, H=8, E=64) on 8 TRN2 cores.

Sharding: 16 (batch, head) pairs, 2 per core (cores 0-3 batch 0, cores 4-7
batch 1). Each core runs the same Bass program on its two pairs.

Math: logits = (Q K^T) * (tau/sqrt(E)) + delta/sqrt(E), causal softmax, A V.
Host folds: Q pre-scaled by tau/sqrt(E); exp(delta/sqrt(E)) folded into V and
into an appended denominator column, since
softmax(x + d)_s = exp(x_s) e^{d_s} / sum_j exp(x_j) e^{d_j}.
The device computes T[s,l] = exp(q'k), AV into [E+1, 512] PSUM banks, and
ships the UNNORMALIZED numerator + denominator row to the host, which divides
and transposes (so no PE transposes / reciprocals on device).

Per (b,h) pair, scores kept transposed (s on partitions), bank-major over 4
l-banks of 512, diagonal group FIRST within each bank so its mask multiply
(on the otherwise-idle GPSIMD engine) hides behind the bank's other groups.
exp is split between the Scalar engine (true exp LUT) and the Vector engine
(Schraudolph bit-trick exp: i32 = A*x + B, bitcast to f32; rel err ~3e-2 on
individual elements, which washes out below 3e-3 in the final output), so the
two engines chew the softmax in parallel.

Q^T is duplicated on both partition halves and K^T packed (even s-tiles on
partitions 0-63, odd on 64-127) so the two k=64 score matmuls of each
512-chunk pair target disjoint PE row-tiles.

All DRAM layouts are [partition, free]-contiguous so every DMA is few large
descriptors (the previous version spent ~12us of ramp on 33ns scatter
descriptors).
"""

import copy
import sys

import numpy as np

try:
    import concourse.bass as bass
except ImportError:  # pragma: no cover
    sys.path.insert(0, "/opt/trn_rl_repo")
    import concourse.bass as bass

import concourse.mybir as mybir
import concourse.tile as tile
from concourse.bass_utils import run_bass_kernel_spmd
from concourse.vector_clock import ScopedClock

B, L, H, E = 2, 2048, 8, 64
N_CORES = 8
PAIRS_PER_CORE = 2
NT = L // 128  # 16 s-tiles
NB = L // 512  # 4 l-banks
SCALE = 1.0 / np.sqrt(np.float32(E))  # 0.125
VW = 128  # v row: 64 values + denominator col + zero pad to 128 (FWL)

f32 = mybir.dt.float32
f32r = mybir.dt.float32r
bf16 = mybir.dt.bfloat16
i32 = mybir.dt.int32
i16 = mybir.dt.int16

# Schraudolph exp in bf16 bits: exp(x) ~= bitcast_bf16(int16(A*x + B)); C
# tuned empirically (numpy grid) for min final error of this problem.
EXP_A = float(np.float32(2.0**7 / np.log(2.0)))
EXP_C = 486411.0 / 65536.0
EXP_B = float(np.float32(127.0 * 2.0**7 - EXP_C))

# ---------------------------------------------------------------------------
# Walrus in this toolchain rejects >1 sync-wait per instruction. Split extra
# waits onto NoOps committed just before the instruction on the same engine.
# ---------------------------------------------------------------------------
_NOP_TEMPLATE = {}


def _make_nop(engine, name):
    if engine not in _NOP_TEMPLATE:
        tmp = bass.Bass()
        _NOP_TEMPLATE[engine] = tmp.engines[engine].nop(nofuse=True).ins
    nop = copy.copy(_NOP_TEMPLATE[engine])
    nop.name = name
    nop.engine = engine
    nop.sync_info = None
    return nop


class SplitWaitTileContext(tile.TileContext):
    _ws_counter = 0

    def _split_waits(self, inst):
        si = inst.sync_info
        if si is None or not si.on_wait or len(si.on_wait) <= 1:
            return []
        if inst.engine == mybir.EngineType.Unassigned:
            return []
        waits = list(si.on_wait)
        inst.sync_info = mybir.SyncInfo(
            on_wait=[waits[0]], on_update=list(si.on_update or [])
        )
        nops = []
        for w in waits[1:]:
            SplitWaitTileContext._ws_counter += 1
            nop = _make_nop(inst.engine, f"I-ws{SplitWaitTileContext._ws_counter}")
            nop.sync_info = mybir.SyncInfo(on_wait=[w], on_update=[])
            nops.append(nop)
        return nops

    def _commit_instruction(self, inst, lazy_reg_writes=True):
        for nop in self._split_waits(inst):
            self._add_instruction(nop)
        super()._commit_instruction(inst, lazy_reg_writes)

    def _drain_and_barrier(self, tick_clock, wait_clock):
        nc = self.nc
        probe = nc.sync.nop(nofuse=True)
        wait_clock.add_sem_waits(
            probe.ins, ScopedClock({None: tick_clock.global_clock})
        )
        waits = list(probe.ins.sync_info.on_wait or []) if probe.ins.sync_info else []
        if len(waits) > 1:
            probe.ins.sync_info.on_wait = [waits[0]]
            handles = {h.num: h for h in self.sems.allocated().values()}
            for w in waits[1:]:
                nop = nc.sync.nop(nofuse=True)
                nop.wait_op(handles[w.id], w.wait_value, "sem-ge")
        nc.sync.drain()

        nc.all_engine_barrier()
        assert self.sems is not None
        popped = nc._tile_sem_poison_stack.pop()
        assert popped is self._sem_poison
        nc.clear_and_free_semaphores(list(self.sems.allocated().values()))
        nc.all_engine_barrier()


# ---------------------------------------------------------------------------
# Program builder
# ---------------------------------------------------------------------------

def build_program(dve_num=5, dve_den=8, mask_engine="gpsimd", st_dtype=bf16):
    """dve_num/dve_den: fraction of off-diagonal exp halves routed to the
    Vector engine's Schraudolph path (rest go to the Scalar engine's LUT)."""
    nc = bass.Bass()
    Exp = mybir.ActivationFunctionType.Exp
    Alu = mybir.AluOpType

    qt = nc.declare_dram_parameter("qt", [PAIRS_PER_CORE, 128, L], st_dtype, isOutput=False)
    kt = nc.declare_dram_parameter("kt", [PAIRS_PER_CORE, 128, L // 2], st_dtype, isOutput=False)
    vv = nc.declare_dram_parameter("vv", [PAIRS_PER_CORE, 128, NT, VW], bf16, isOutput=False)
    mask = nc.declare_dram_parameter("mask", [128, 128], bf16, isOutput=False)
    oo = nc.declare_dram_parameter("oo", [PAIRS_PER_CORE, E + 1, L], f32, isOutput=True)

    with SplitWaitTileContext(nc) as tc:
        with (
            tc.tile_pool(name="const", bufs=1) as constp,
            tc.tile_pool(name="qk", bufs=2) as qkp,
            tc.tile_pool(name="vp", bufs=2) as vp,
            tc.tile_pool(name="ap", bufs=6) as ap_pool,
            tc.tile_pool(name="outp", bufs=2) as outp,
            tc.tile_pool(name="st", bufs=1, space="PSUM") as stp,
            tc.tile_pool(name="otp", bufs=2, space="PSUM") as otp,
        ):
            mask_sb = constp.tile([128, 128], bf16, tag="mask")

            # ---- all input loads up front (sync-engine HWDGE FIFO) ----
            qt_sbs, kt_sbs, v_sbs = [], [], []
            for pair in range(PAIRS_PER_CORE):
                qt_sbs.append(qkp.tile([128, L], st_dtype, tag="qt", name="qt_sb"))
                kt_sbs.append(qkp.tile([128, L // 2], st_dtype, tag="kt", name="kt_sb"))
                v_sbs.append(vp.tile([128, NT, VW], bf16, tag="v", name="v_sb"))
            for pair in range(PAIRS_PER_CORE):
                for ch in range(4):
                    cl = slice(512 * ch, 512 * (ch + 1))
                    kl = slice(256 * ch, 256 * (ch + 1))
                    nc.sync.dma_start(out=kt_sbs[pair][:, kl], in_=kt[pair][:, kl])
                    nc.sync.dma_start(out=qt_sbs[pair][:, cl], in_=qt[pair][:, cl])
                    nc.sync.dma_start(
                        out=v_sbs[pair][:, 4 * ch : 4 * ch + 4, :],
                        in_=vv[pair][:, 4 * ch : 4 * ch + 4, :],
                    )
                    if pair == 0 and ch == 0:
                        nc.sync.dma_start(out=mask_sb, in_=mask[:])

            st_slot = [0]
            # round-robin splitter for off-diagonal exp halves
            dve_acc = [0]

            def use_dve():
                dve_acc[0] += dve_num
                if dve_acc[0] >= dve_den:
                    dve_acc[0] -= dve_den
                    return True
                return False

            mask_eng = {"gpsimd": nc.gpsimd, "vector": nc.vector}[mask_engine]

            out_sbs = [
                outp.tile([E + 1, L], f32, tag="out", name="out_sb")
                for pair in range(PAIRS_PER_CORE)
            ]
            ot_banks = {}

            if True:
                def emit_st_group(pair, lj, gi):
                    qt_sb, kt_sb = qt_sbs[pair], kt_sbs[pair]
                    diag = gi == lj
                    a_grp = ap_pool.tile([128, 2048], bf16, tag="A", name="A")
                    for hb in range(2):
                        st_slot[0] = (st_slot[0] + 1) % 3
                        st = stp.tile(
                            [128, 1024], f32, tag=f"st{st_slot[0]}", name="st"
                        )
                        for cc in range(2):
                            c = 2 * hb + cc
                            si = 4 * gi + c
                            off = 128 * c if diag else 0
                            half = (si % 2) * E
                            kcol = (si // 2) * 128
                            nc.tensor.matmul(
                                st[:, 512 * cc + off : 512 * (cc + 1)],
                                kt_sb[half : half + E, kcol : kcol + 128],
                                qt_sb[half : half + E, 512 * lj + off : 512 * lj + 512],
                                start=True,
                                stop=True,
                            )
                        if diag:
                            for cc in range(2):
                                c = 2 * hb + cc
                                off = 128 * c
                                nc.scalar.activation(
                                    out=a_grp[:, 512 * c + off : 512 * (c + 1)],
                                    in_=st[:, 512 * cc + off : 512 * (cc + 1)],
                                    func=Exp,
                                    scale=1.0,
                                )
                        elif use_dve():
                            nc.vector.tensor_scalar(
                                a_grp[:, 1024 * hb : 1024 * (hb + 1)].bitcast(i16),
                                st[:, :],
                                EXP_A,
                                EXP_B,
                                Alu.mult,
                                Alu.add,
                            )
                        else:
                            nc.scalar.activation(
                                out=a_grp[:, 1024 * hb : 1024 * (hb + 1)],
                                in_=st[:, :],
                                func=Exp,
                                scale=1.0,
                            )
                    if diag:
                        for c in range(4):
                            colb = 512 * c + 128 * c
                            mask_eng.tensor_mul(
                                a_grp[:, colb : colb + 128],
                                a_grp[:, colb : colb + 128],
                                mask_sb,
                            )
                    return a_grp

                def emit_av_group(pair, lj, gi, a_grp, first, last):
                    diag = gi == lj
                    v_sb = v_sbs[pair]
                    ot = ot_banks[(pair, lj)]
                    for c in range(4):
                        si = 4 * gi + c
                        off = 128 * c if diag else 0
                        nc.tensor.matmul(
                            ot[:, off:512],
                            v_sb[:, si, :],
                            a_grp[:, 512 * c + off : 512 * (c + 1)],
                            start=(first and c == 0),
                            stop=(last and c == 3),
                        )

                def epilogue(pair, lj):
                    ot = ot_banks.pop((pair, lj))
                    cl = slice(512 * lj, 512 * (lj + 1))
                    nc.vector.tensor_copy(out_sbs[pair][:, cl], ot[0 : E + 1, :])
                    nc.sync.dma_start(
                        out=oo[pair][:, cl], in_=out_sbs[pair][:, cl]
                    )

                # ST order: the two pairs' group streams interleaved,
                # bank-major, diagonal group first within each bank (so its
                # gpsimd mask-mult hides behind the bank's other groups).
                # AV order: diagonal group LAST per bank (start flag on the
                # first-issued AV of each bank, stop on the diagonal).
                st_order = []
                av_order = []
                for lj in range(NB):
                    for pair in range(PAIRS_PER_CORE):
                        st_order.append((pair, lj, lj))
                        st_order.extend((pair, lj, gi) for gi in range(lj))
                        av_order.extend((pair, lj, gi) for gi in range(lj))
                        av_order.append((pair, lj, lj))
                st_slot_of = {g: k for k, g in enumerate(st_order)}
                a_cache = {}
                av_k = 0

                def emit_av(item):
                    pair, alj, agi = item
                    emit_av_group(
                        pair, alj, agi, a_cache.pop(item),
                        first=(agi == 0 if alj > 0 else True),
                        last=(agi == alj),
                    )
                    if agi == alj:
                        epilogue(pair, alj)

                for st_k, (pair, lj, gi) in enumerate(st_order):
                    if (pair, lj) not in ot_banks:
                        ot_banks[(pair, lj)] = otp.tile(
                            [128, 512], f32, tag="ot", name="ot"
                        )
                    a_cache[(pair, lj, gi)] = emit_st_group(pair, lj, gi)
                    while av_k < len(av_order) and av_order[av_k] in a_cache \
                            and st_slot_of[av_order[av_k]] < st_k:
                        emit_av(av_order[av_k])
                        av_k += 1
                while av_k < len(av_order):
                    emit_av(av_order[av_k])
                    av_k += 1

    return nc


# ---------------------------------------------------------------------------
# Host-side sharding / unsharding
# ---------------------------------------------------------------------------

def _in_maps(queries, keys, values, tau, delta, st_dtype=bf16):
    np_st = mybir.dt.np(st_dtype)
    np_bf = mybir.dt.np(bf16)
    mask = np.triu(np.ones((128, 128), dtype=np.float32)).astype(np_bf)
    maps = []
    for c in range(N_CORES):
        ps = [2 * c, 2 * c + 1]
        b = ps[0] // H
        hs = [p % H for p in ps]
        qscale = np.float32(SCALE * tau[b, 0])
        expd = np.exp(SCALE * delta[b]).astype(np.float32)  # [L]
        qt = np.empty((PAIRS_PER_CORE, 128, L), dtype=np_st)
        kt = np.empty((PAIRS_PER_CORE, 128, L // 2), dtype=np_st)
        vv = np.zeros((PAIRS_PER_CORE, 128, NT, VW), dtype=np_bf)
        for i, h in enumerate(hs):
            qT = (queries[b, :, h, :].T * qscale).astype(np_st)  # [E, L]
            qt[i, 0:E] = qT
            qt[i, E:2 * E] = qT
            kT = keys[b, :, h, :].T.astype(np_st)  # [E, L]
            ktile = kT.reshape(E, NT, 128)
            kt[i, 0:E] = np.ascontiguousarray(
                ktile[:, 0::2, :]).reshape(E, L // 2)
            kt[i, E:2 * E] = np.ascontiguousarray(
                ktile[:, 1::2, :]).reshape(E, L // 2)
            vaug = np.zeros((L, VW), dtype=np.float32)
            vaug[:, 0:E] = values[b, :, h, :] * expd[:, None]
            vaug[:, E] = expd
            vv[i] = vaug.reshape(NT, 128, VW).transpose(1, 0, 2).astype(np_bf)
        maps.append({"qt": qt, "kt": kt, "vv": vv, "mask": mask})
    return maps


_CACHED = {}


def run(queries, keys, values, tau, delta, trace=False, st_dtype=bf16,
        av_dtype=None, dve_num=5, dve_den=8, mask_engine="gpsimd"):
    key = (str(st_dtype), dve_num, dve_den, mask_engine)
    if key not in _CACHED:
        _CACHED[key] = build_program(
            dve_num=dve_num, dve_den=dve_den, mask_engine=mask_engine,
            st_dtype=st_dtype)
    nc = _CACHED[key]
    in_maps = _in_maps(
        np.asarray(queries),
        np.asarray(keys),
        np.asarray(values),
        np.asarray(tau),
        np.asarray(delta),
        st_dtype=st_dtype,
    )
    res = run_bass_kernel_spmd(
        nc, in_maps, core_ids=list(range(N_CORES)), trace=trace
    )
    out = np.empty((B, L, H, E), dtype=np.float32)
    for c in range(N_CORES):
        o = res.results[c]["oo"]  # [2, E+1, L]
        for i, p in enumerate([2 * c, 2 * c + 1]):
            out[p // H, :, p % H, :] = (o[i, 0:E, :] / o[i, E, :]).T
    return out, res


def kernel(queries, keys, values, tau, delta):
    out, _ = run(queries, keys, values, tau, delta, trace=False)
    return out


# revision 9
# speedup vs baseline: 1.2813x; 1.2813x over previous
"""De-stationary causal attention (B=2, L=S=2048, H=8, E=64) on 8 TRN2 cores.

Sharding: 16 (batch, head) pairs, 2 per core (cores 0-3 batch 0, cores 4-7
batch 1). Each core runs the same Bass program on its two pairs.

Math: logits = (Q K^T) * (tau/sqrt(E)) + delta/sqrt(E), causal softmax, A V.
Host folds: Q pre-scaled by tau/sqrt(E); exp(delta/sqrt(E)) folded into V and
into an appended denominator column, since
softmax(x + d)_s = exp(x_s) e^{d_s} / sum_j exp(x_j) e^{d_j}.
The device computes T[s,l] = exp(q'k), AV into [E+1, 512] PSUM banks, and
ships the UNNORMALIZED numerator + denominator row to the host, which divides
and transposes (so no PE transposes / reciprocals on device).

Per (b,h) pair, scores kept transposed (s on partitions), bank-major over 4
l-banks of 512, diagonal group FIRST within each bank so its mask multiply
(on the otherwise-idle GPSIMD engine) hides behind the bank's other groups.
exp is split between the Scalar engine (true exp LUT) and the Vector engine
(Schraudolph bit-trick exp: i32 = A*x + B, bitcast to f32; rel err ~3e-2 on
individual elements, which washes out below 3e-3 in the final output), so the
two engines chew the softmax in parallel.

Q^T is duplicated on both partition halves and K^T packed (even s-tiles on
partitions 0-63, odd on 64-127) so the two k=64 score matmuls of each
512-chunk pair target disjoint PE row-tiles.

All DRAM layouts are [partition, free]-contiguous so every DMA is few large
descriptors (the previous version spent ~12us of ramp on 33ns scatter
descriptors).
"""

import copy
import sys

import numpy as np

try:
    import concourse.bass as bass
except ImportError:  # pragma: no cover
    sys.path.insert(0, "/opt/trn_rl_repo")
    import concourse.bass as bass

import concourse.mybir as mybir
import concourse.tile as tile
from concourse.bass_utils import run_bass_kernel_spmd
from concourse.vector_clock import ScopedClock

B, L, H, E = 2, 2048, 8, 64
N_CORES = 8
PAIRS_PER_CORE = 2
NT = L // 128  # 16 s-tiles
NB = L // 512  # 4 l-banks
SCALE = 1.0 / np.sqrt(np.float32(E))  # 0.125
VW = 128  # v row: 64 values + denominator col + zero pad to 128 (FWL)

f32 = mybir.dt.float32
f32r = mybir.dt.float32r
bf16 = mybir.dt.bfloat16
i32 = mybir.dt.int32
i16 = mybir.dt.int16

# Schraudolph exp in bf16 bits: exp(x) ~= bitcast_bf16(int16(A*x + B)); C
# tuned empirically (numpy grid) for min final error of this problem.
EXP_A = float(np.float32(2.0**7 / np.log(2.0)))
EXP_C = 486411.0 / 65536.0
EXP_B = float(np.float32(127.0 * 2.0**7 - EXP_C))

# ---------------------------------------------------------------------------
# Walrus in this toolchain rejects >1 sync-wait per instruction. Split extra
# waits onto NoOps committed just before the instruction on the same engine.
# ---------------------------------------------------------------------------
_NOP_TEMPLATE = {}


def _make_nop(engine, name):
    if engine not in _NOP_TEMPLATE:
        tmp = bass.Bass()
        _NOP_TEMPLATE[engine] = tmp.engines[engine].nop(nofuse=True).ins
    nop = copy.copy(_NOP_TEMPLATE[engine])
    nop.name = name
    nop.engine = engine
    nop.sync_info = None
    return nop


class SplitWaitTileContext(tile.TileContext):
    _ws_counter = 0

    def _split_waits(self, inst):
        si = inst.sync_info
        if si is None or not si.on_wait or len(si.on_wait) <= 1:
            return []
        if inst.engine == mybir.EngineType.Unassigned:
            return []
        waits = list(si.on_wait)
        inst.sync_info = mybir.SyncInfo(
            on_wait=[waits[0]], on_update=list(si.on_update or [])
        )
        nops = []
        for w in waits[1:]:
            SplitWaitTileContext._ws_counter += 1
            nop = _make_nop(inst.engine, f"I-ws{SplitWaitTileContext._ws_counter}")
            nop.sync_info = mybir.SyncInfo(on_wait=[w], on_update=[])
            nops.append(nop)
        return nops

    def _commit_instruction(self, inst, lazy_reg_writes=True):
        for nop in self._split_waits(inst):
            self._add_instruction(nop)
        super()._commit_instruction(inst, lazy_reg_writes)

    def _drain_and_barrier(self, tick_clock, wait_clock):
        nc = self.nc
        probe = nc.sync.nop(nofuse=True)
        wait_clock.add_sem_waits(
            probe.ins, ScopedClock({None: tick_clock.global_clock})
        )
        waits = list(probe.ins.sync_info.on_wait or []) if probe.ins.sync_info else []
        if len(waits) > 1:
            probe.ins.sync_info.on_wait = [waits[0]]
            handles = {h.num: h for h in self.sems.allocated().values()}
            for w in waits[1:]:
                nop = nc.sync.nop(nofuse=True)
                nop.wait_op(handles[w.id], w.wait_value, "sem-ge")
        nc.sync.drain()

        nc.all_engine_barrier()
        assert self.sems is not None
        popped = nc._tile_sem_poison_stack.pop()
        assert popped is self._sem_poison
        nc.clear_and_free_semaphores(list(self.sems.allocated().values()))
        nc.all_engine_barrier()


# ---------------------------------------------------------------------------
# Program builder
# ---------------------------------------------------------------------------

def build_program(dve_num=5, dve_den=8, mask_engine="gpsimd", st_dtype=bf16):
    """dve_num/dve_den: fraction of off-diagonal exp halves routed to the
    Vector engine's Schraudolph path (rest go to the Scalar engine's LUT)."""
    nc = bass.Bass()
    Exp = mybir.ActivationFunctionType.Exp
    Alu = mybir.AluOpType

    qt = nc.declare_dram_parameter("qt", [PAIRS_PER_CORE, 128, L], st_dtype, isOutput=False)
    kt = nc.declare_dram_parameter("kt", [PAIRS_PER_CORE, 128, L // 2], st_dtype, isOutput=False)
    vv = nc.declare_dram_parameter("vv", [PAIRS_PER_CORE, 128, NT, VW], bf16, isOutput=False)
    mask = nc.declare_dram_parameter("mask", [128, 128], bf16, isOutput=False)
    oo = nc.declare_dram_parameter("oo", [PAIRS_PER_CORE, E + 1, L], f32, isOutput=True)

    with SplitWaitTileContext(nc) as tc:
        with (
            tc.tile_pool(name="const", bufs=1) as constp,
            tc.tile_pool(name="qk", bufs=2) as qkp,
            tc.tile_pool(name="vp", bufs=2) as vp,
            tc.tile_pool(name="ap", bufs=6) as ap_pool,
            tc.tile_pool(name="outp", bufs=2) as outp,
            tc.tile_pool(name="st", bufs=1, space="PSUM") as stp,
            tc.tile_pool(name="otp", bufs=2, space="PSUM") as otp,
        ):
            mask_sb = constp.tile([128, 128], bf16, tag="mask")

            # ---- all input loads up front (sync-engine HWDGE FIFO) ----
            qt_sbs, kt_sbs, v_sbs = [], [], []
            for pair in range(PAIRS_PER_CORE):
                qt_sbs.append(qkp.tile([128, L], st_dtype, tag="qt", name="qt_sb"))
                kt_sbs.append(qkp.tile([128, L // 2], st_dtype, tag="kt", name="kt_sb"))
                v_sbs.append(vp.tile([128, NT, VW], bf16, tag="v", name="v_sb"))
            def load_qk(pair, ch):
                cl = slice(512 * ch, 512 * (ch + 1))
                kl = slice(256 * ch, 256 * (ch + 1))
                nc.sync.dma_start(out=kt_sbs[pair][:, kl], in_=kt[pair][:, kl])
                nc.sync.dma_start(out=qt_sbs[pair][:, cl], in_=qt[pair][:, cl])

            def load_v(pair, ch):
                nc.sync.dma_start(
                    out=v_sbs[pair][:, 4 * ch : 4 * ch + 4, :],
                    in_=vv[pair][:, 4 * ch : 4 * ch + 4, :],
                )

            load_qk(0, 0)
            load_qk(0, 1)
            load_v(0, 0)
            nc.sync.dma_start(out=mask_sb, in_=mask[:])
            for ch in (2, 3):
                load_qk(0, ch)
            for ch in (1, 2, 3):
                load_v(0, ch)
            for ch in range(4):
                load_qk(1, ch)
            for ch in range(4):
                load_v(1, ch)

            st_slot = [0]
            # round-robin splitter for off-diagonal exp halves
            dve_acc = [0]

            def use_dve():
                dve_acc[0] += dve_num
                if dve_acc[0] >= dve_den:
                    dve_acc[0] -= dve_den
                    return True
                return False

            mask_eng = {"gpsimd": nc.gpsimd, "vector": nc.vector}[mask_engine]

            for pair in range(PAIRS_PER_CORE):
                qt_sb, kt_sb, v_sb = qt_sbs[pair], kt_sbs[pair], v_sbs[pair]
                out_sb = outp.tile([E + 1, L], f32, tag="out")
                ot_banks = {}

                def emit_st_group(lj, gi):
                    diag = gi == lj
                    a_grp = ap_pool.tile([128, 2048], bf16, tag="A", name="A")
                    for hb in range(2):
                        st_slot[0] = (st_slot[0] + 1) % 3
                        st = stp.tile(
                            [128, 1024], f32, tag=f"st{st_slot[0]}", name="st"
                        )
                        for cc in range(2):
                            c = 2 * hb + cc
                            si = 4 * gi + c
                            off = 128 * c if diag else 0
                            half = (si % 2) * E
                            kcol = (si // 2) * 128
                            nc.tensor.matmul(
                                st[:, 512 * cc + off : 512 * (cc + 1)],
                                kt_sb[half : half + E, kcol : kcol + 128],
                                qt_sb[half : half + E, 512 * lj + off : 512 * lj + 512],
                                start=True,
                                stop=True,
                            )
                        if diag:
                            for cc in range(2):
                                c = 2 * hb + cc
                                off = 128 * c
                                nc.scalar.activation(
                                    out=a_grp[:, 512 * c + off : 512 * (c + 1)],
                                    in_=st[:, 512 * cc + off : 512 * (cc + 1)],
                                    func=Exp,
                                    scale=1.0,
                                )
                        elif use_dve():
                            nc.vector.tensor_scalar(
                                a_grp[:, 1024 * hb : 1024 * (hb + 1)].bitcast(i16),
                                st[:, :],
                                EXP_A,
                                EXP_B,
                                Alu.mult,
                                Alu.add,
                            )
                        else:
                            nc.scalar.activation(
                                out=a_grp[:, 1024 * hb : 1024 * (hb + 1)],
                                in_=st[:, :],
                                func=Exp,
                                scale=1.0,
                            )
                    if diag:
                        for c in range(4):
                            colb = 512 * c + 128 * c
                            mask_eng.tensor_mul(
                                a_grp[:, colb : colb + 128],
                                a_grp[:, colb : colb + 128],
                                mask_sb,
                            )
                    return a_grp

                def emit_av_group(lj, gi, a_grp, first, last):
                    diag = gi == lj
                    ot = ot_banks[lj]
                    for c in range(4):
                        si = 4 * gi + c
                        off = 128 * c if diag else 0
                        nc.tensor.matmul(
                            ot[:, off:512],
                            v_sb[:, si, :],
                            a_grp[:, 512 * c + off : 512 * (c + 1)],
                            start=(first and c == 0),
                            stop=(last and c == 3),
                        )

                def epilogue(lj):
                    ot = ot_banks.pop(lj)
                    cl = slice(512 * lj, 512 * (lj + 1))
                    nc.vector.tensor_copy(out_sb[:, cl], ot[0 : E + 1, :])
                    nc.sync.dma_start(out=oo[pair][:, cl], in_=out_sb[:, cl])

                # ST order: bank-major, diagonal group first within each bank
                # (so its gpsimd mask-mult hides behind the bank's other
                # groups). AV order: diagonal group LAST (start flag on the
                # first-issued AV of each bank, stop on the diagonal).
                st_order = []
                av_order = []
                for lj in range(NB):
                    st_order.append((lj, lj))
                    st_order.extend((lj, gi) for gi in range(lj))
                    av_order.extend((lj, gi) for gi in range(lj))
                    av_order.append((lj, lj))
                a_cache = {}
                av_k = 0
                for st_k, (lj, gi) in enumerate(st_order):
                    if lj not in ot_banks:
                        ot_banks[lj] = otp.tile(
                            [128, 512], f32, tag="ot", name="ot"
                        )
                    a_cache[(lj, gi)] = emit_st_group(lj, gi)
                    # emit AV for any group whose ST was emitted in a
                    # previous slot (1-slot software pipeline lag)
                    while av_k < len(av_order) and av_order[av_k] in a_cache and \
                            st_order.index(av_order[av_k]) < st_k:
                        alj, agi = av_order[av_k]
                        emit_av_group(
                            alj, agi, a_cache.pop((alj, agi)),
                            first=(agi == 0 if alj > 0 else True),
                            last=(agi == alj),
                        )
                        if agi == alj:
                            epilogue(alj)
                        av_k += 1
                while av_k < len(av_order):
                    alj, agi = av_order[av_k]
                    emit_av_group(
                        alj, agi, a_cache.pop((alj, agi)),
                        first=(agi == 0 if alj > 0 else True),
                        last=(agi == alj),
                    )
                    if agi == alj:
                        epilogue(alj)
                    av_k += 1

    return nc


# ---------------------------------------------------------------------------
# Host-side sharding / unsharding
# ---------------------------------------------------------------------------

def _in_maps(queries, keys, values, tau, delta, st_dtype=bf16):
    np_st = mybir.dt.np(st_dtype)
    np_bf = mybir.dt.np(bf16)
    mask = np.triu(np.ones((128, 128), dtype=np.float32)).astype(np_bf)
    maps = []
    for c in range(N_CORES):
        ps = [2 * c, 2 * c + 1]
        b = ps[0] // H
        hs = [p % H for p in ps]
        qscale = np.float32(SCALE * tau[b, 0])
        expd = np.exp(SCALE * delta[b]).astype(np.float32)  # [L]
        qt = np.empty((PAIRS_PER_CORE, 128, L), dtype=np_st)
        kt = np.empty((PAIRS_PER_CORE, 128, L // 2), dtype=np_st)
        vv = np.zeros((PAIRS_PER_CORE, 128, NT, VW), dtype=np_bf)
        for i, h in enumerate(hs):
            qT = (queries[b, :, h, :].T * qscale).astype(np_st)  # [E, L]
            qt[i, 0:E] = qT
            qt[i, E:2 * E] = qT
            kT = keys[b, :, h, :].T.astype(np_st)  # [E, L]
            ktile = kT.reshape(E, NT, 128)
            kt[i, 0:E] = np.ascontiguousarray(
                ktile[:, 0::2, :]).reshape(E, L // 2)
            kt[i, E:2 * E] = np.ascontiguousarray(
                ktile[:, 1::2, :]).reshape(E, L // 2)
            vaug = np.zeros((L, VW), dtype=np.float32)
            vaug[:, 0:E] = values[b, :, h, :] * expd[:, None]
            vaug[:, E] = expd
            vv[i] = vaug.reshape(NT, 128, VW).transpose(1, 0, 2).astype(np_bf)
        maps.append({"qt": qt, "kt": kt, "vv": vv, "mask": mask})
    return maps


_CACHED = {}


def run(queries, keys, values, tau, delta, trace=False, st_dtype=bf16,
        av_dtype=None, dve_num=5, dve_den=8, mask_engine="gpsimd"):
    key = (str(st_dtype), dve_num, dve_den, mask_engine)
    if key not in _CACHED:
        _CACHED[key] = build_program(
            dve_num=dve_num, dve_den=dve_den, mask_engine=mask_engine,
            st_dtype=st_dtype)
    nc = _CACHED[key]
    in_maps = _in_maps(
        np.asarray(queries),
        np.asarray(keys),
        np.asarray(values),
        np.asarray(tau),
        np.asarray(delta),
        st_dtype=st_dtype,
    )
    res = run_bass_kernel_spmd(
        nc, in_maps, core_ids=list(range(N_CORES)), trace=trace
    )
    out = np.empty((B, L, H, E), dtype=np.float32)
    for c in range(N_CORES):
        o = res.results[c]["oo"]  # [2, E+1, L]
        for i, p in enumerate([2 * c, 2 * c + 1]):
            out[p // H, :, p % H, :] = (o[i, 0:E, :] / o[i, E, :]).T
    return out, res


def kernel(queries, keys, values, tau, delta):
    out, _ = run(queries, keys, values, tau, delta, trace=False)
    return out


# revision 10
# speedup vs baseline: 1.3402x; 1.0460x over previous
"""De-stationary causal attention (B=2, L=S=2048, H=8, E=64) on 8 TRN2 cores.

Sharding: 16 (batch, head) pairs, 2 per core (cores 0-3 batch 0, cores 4-7
batch 1). Each core runs the same Bass program on its two pairs.

Math: logits = (Q K^T) * (tau/sqrt(E)) + delta/sqrt(E), causal softmax, A V.
Host folds: Q pre-scaled by tau/sqrt(E); exp(delta/sqrt(E)) folded into V and
into an appended denominator column, since
softmax(x + d)_s = exp(x_s) e^{d_s} / sum_j exp(x_j) e^{d_j}.
The device computes T[s,l] = exp(q'k), AV into [E+1, 512] PSUM banks, and
ships the UNNORMALIZED numerator + denominator row to the host, which divides
and transposes (so no PE transposes / reciprocals on device).

Per (b,h) pair, scores kept transposed (s on partitions), bank-major over 4
l-banks of 512, diagonal group FIRST within each bank so its mask multiply
(on the otherwise-idle GPSIMD engine) hides behind the bank's other groups.
exp is split between the Scalar engine (true exp LUT) and the Vector engine
(Schraudolph bit-trick exp: i32 = A*x + B, bitcast to f32; rel err ~3e-2 on
individual elements, which washes out below 3e-3 in the final output), so the
two engines chew the softmax in parallel.

Q^T is duplicated on both partition halves and K^T packed (even s-tiles on
partitions 0-63, odd on 64-127) so the two k=64 score matmuls of each
512-chunk pair target disjoint PE row-tiles.

All DRAM layouts are [partition, free]-contiguous so every DMA is few large
descriptors (the previous version spent ~12us of ramp on 33ns scatter
descriptors).
"""

import copy
import sys

import numpy as np

try:
    import concourse.bass as bass
except ImportError:  # pragma: no cover
    sys.path.insert(0, "/opt/trn_rl_repo")
    import concourse.bass as bass

import concourse.mybir as mybir
import concourse.tile as tile
from concourse.bass_utils import run_bass_kernel_spmd
from concourse.vector_clock import ScopedClock

B, L, H, E = 2, 2048, 8, 64
N_CORES = 8
PAIRS_PER_CORE = 2
NT = L // 128  # 16 s-tiles
NB = L // 512  # 4 l-banks
SCALE = 1.0 / np.sqrt(np.float32(E))  # 0.125
VW = 128  # v row: 64 values + denominator col + zero pad to 128 (FWL)

f32 = mybir.dt.float32
f32r = mybir.dt.float32r
bf16 = mybir.dt.bfloat16
i32 = mybir.dt.int32
i16 = mybir.dt.int16

# Schraudolph exp in bf16 bits: exp(x) ~= bitcast_bf16(int16(A*x + B)); C
# tuned empirically (numpy grid) for min final error of this problem.
EXP_A = float(np.float32(2.0**7 / np.log(2.0)))
EXP_C = 486411.0 / 65536.0
EXP_B = float(np.float32(127.0 * 2.0**7 - EXP_C))

# ---------------------------------------------------------------------------
# Walrus in this toolchain rejects >1 sync-wait per instruction. Split extra
# waits onto NoOps committed just before the instruction on the same engine.
# ---------------------------------------------------------------------------
_NOP_TEMPLATE = {}


def _make_nop(engine, name):
    if engine not in _NOP_TEMPLATE:
        tmp = bass.Bass()
        _NOP_TEMPLATE[engine] = tmp.engines[engine].nop(nofuse=True).ins
    nop = copy.copy(_NOP_TEMPLATE[engine])
    nop.name = name
    nop.engine = engine
    nop.sync_info = None
    return nop


class SplitWaitTileContext(tile.TileContext):
    _ws_counter = 0

    def _split_waits(self, inst):
        si = inst.sync_info
        if si is None or not si.on_wait or len(si.on_wait) <= 1:
            return []
        if inst.engine == mybir.EngineType.Unassigned:
            return []
        waits = list(si.on_wait)
        inst.sync_info = mybir.SyncInfo(
            on_wait=[waits[0]], on_update=list(si.on_update or [])
        )
        nops = []
        for w in waits[1:]:
            SplitWaitTileContext._ws_counter += 1
            nop = _make_nop(inst.engine, f"I-ws{SplitWaitTileContext._ws_counter}")
            nop.sync_info = mybir.SyncInfo(on_wait=[w], on_update=[])
            nops.append(nop)
        return nops

    def _commit_instruction(self, inst, lazy_reg_writes=True):
        for nop in self._split_waits(inst):
            self._add_instruction(nop)
        super()._commit_instruction(inst, lazy_reg_writes)

    def _drain_and_barrier(self, tick_clock, wait_clock):
        nc = self.nc
        probe = nc.sync.nop(nofuse=True)
        wait_clock.add_sem_waits(
            probe.ins, ScopedClock({None: tick_clock.global_clock})
        )
        waits = list(probe.ins.sync_info.on_wait or []) if probe.ins.sync_info else []
        if len(waits) > 1:
            probe.ins.sync_info.on_wait = [waits[0]]
            handles = {h.num: h for h in self.sems.allocated().values()}
            for w in waits[1:]:
                nop = nc.sync.nop(nofuse=True)
                nop.wait_op(handles[w.id], w.wait_value, "sem-ge")
        nc.sync.drain()

        nc.all_engine_barrier()
        assert self.sems is not None
        popped = nc._tile_sem_poison_stack.pop()
        assert popped is self._sem_poison
        nc.clear_and_free_semaphores(list(self.sems.allocated().values()))
        nc.all_engine_barrier()


# ---------------------------------------------------------------------------
# Program builder
# ---------------------------------------------------------------------------

def build_program(dve_num=5, dve_den=8, mask_engine="gpsimd", st_dtype=bf16):
    """dve_num/dve_den: fraction of off-diagonal exp halves routed to the
    Vector engine's Schraudolph path (rest go to the Scalar engine's LUT)."""
    nc = bass.Bass()
    Exp = mybir.ActivationFunctionType.Exp
    Alu = mybir.AluOpType

    qt = nc.declare_dram_parameter("qt", [PAIRS_PER_CORE, 128, L], st_dtype, isOutput=False)
    kt = nc.declare_dram_parameter("kt", [PAIRS_PER_CORE, 128, L // 2], st_dtype, isOutput=False)
    vv = nc.declare_dram_parameter("vv", [PAIRS_PER_CORE, 128, NT, VW], bf16, isOutput=False)
    mask = nc.declare_dram_parameter("mask", [128, 128], bf16, isOutput=False)
    oo = nc.declare_dram_parameter("oo", [PAIRS_PER_CORE, E + 1, L], f32, isOutput=True)

    with SplitWaitTileContext(nc) as tc:
        with (
            tc.tile_pool(name="const", bufs=1) as constp,
            tc.tile_pool(name="qk", bufs=2) as qkp,
            tc.tile_pool(name="vp", bufs=2) as vp,
            tc.tile_pool(name="ap", bufs=6) as ap_pool,
            tc.tile_pool(name="outp", bufs=2) as outp,
            tc.tile_pool(name="st", bufs=1, space="PSUM") as stp,
            tc.tile_pool(name="otp", bufs=2, space="PSUM") as otp,
        ):
            mask_sb = constp.tile([128, 128], bf16, tag="mask")

            # ---- all input loads up front (sync-engine HWDGE FIFO) ----
            qt_sbs, kt_sbs, v_sbs = [], [], []
            for pair in range(PAIRS_PER_CORE):
                qt_sbs.append(qkp.tile([128, L], st_dtype, tag="qt", name="qt_sb"))
                kt_sbs.append(qkp.tile([128, L // 2], st_dtype, tag="kt", name="kt_sb"))
                v_sbs.append(vp.tile([128, NT, VW], bf16, tag="v", name="v_sb"))
            def load_qk(pair, ch):
                cl = slice(512 * ch, 512 * (ch + 1))
                kl = slice(256 * ch, 256 * (ch + 1))
                nc.sync.dma_start(out=kt_sbs[pair][:, kl], in_=kt[pair][:, kl])
                nc.sync.dma_start(out=qt_sbs[pair][:, cl], in_=qt[pair][:, cl])

            def load_v(pair, ch):
                nc.sync.dma_start(
                    out=v_sbs[pair][:, 4 * ch : 4 * ch + 4, :],
                    in_=vv[pair][:, 4 * ch : 4 * ch + 4, :],
                )

            load_qk(0, 0)
            load_qk(0, 1)
            load_v(0, 0)
            nc.sync.dma_start(out=mask_sb, in_=mask[:])
            for ch in (2, 3):
                load_qk(0, ch)
            for ch in (1, 2, 3):
                load_v(0, ch)
            for ch in range(4):
                load_qk(1, ch)
            for ch in range(4):
                load_v(1, ch)

            st_slot = [0]
            # round-robin splitter for off-diagonal exp halves
            dve_acc = [0]

            def use_dve():
                dve_acc[0] += dve_num
                if dve_acc[0] >= dve_den:
                    dve_acc[0] -= dve_den
                    return True
                return False

            mask_eng = {"gpsimd": nc.gpsimd, "vector": nc.vector}[mask_engine]

            for pair in range(PAIRS_PER_CORE):
                qt_sb, kt_sb, v_sb = qt_sbs[pair], kt_sbs[pair], v_sbs[pair]
                out_sb = outp.tile([E + 1, L], f32, tag="out")
                ot_banks = {}

                def emit_st_group(lj, gi):
                    diag = gi == lj
                    a_grp = ap_pool.tile([128, 2048], bf16, tag="A", name="A")
                    for hb in range(2):
                        st_slot[0] = (st_slot[0] + 1) % 3
                        st = stp.tile(
                            [128, 1024], f32, tag=f"st{st_slot[0]}", name="st"
                        )
                        for cc in range(2):
                            c = 2 * hb + cc
                            si = 4 * gi + c
                            off = 128 * c if diag else 0
                            half = (si % 2) * E
                            kcol = (si // 2) * 128
                            nc.tensor.matmul(
                                st[:, 512 * cc + off : 512 * (cc + 1)],
                                kt_sb[half : half + E, kcol : kcol + 128],
                                qt_sb[half : half + E, 512 * lj + off : 512 * lj + 512],
                                start=True,
                                stop=True,
                            )
                        if diag:
                            for cc in range(2):
                                c = 2 * hb + cc
                                off = 128 * c
                                nc.scalar.activation(
                                    out=a_grp[:, 512 * c + off : 512 * (c + 1)],
                                    in_=st[:, 512 * cc + off : 512 * (cc + 1)],
                                    func=Exp,
                                    scale=1.0,
                                )
                        elif use_dve():
                            nc.vector.tensor_scalar(
                                a_grp[:, 1024 * hb : 1024 * (hb + 1)].bitcast(i16),
                                st[:, :],
                                EXP_A,
                                EXP_B,
                                Alu.mult,
                                Alu.add,
                            )
                        else:
                            nc.scalar.activation(
                                out=a_grp[:, 1024 * hb : 1024 * (hb + 1)],
                                in_=st[:, :],
                                func=Exp,
                                scale=1.0,
                            )
                    if diag:
                        for c in range(4):
                            colb = 512 * c + 128 * c
                            mask_eng.tensor_mul(
                                a_grp[:, colb : colb + 128],
                                a_grp[:, colb : colb + 128],
                                mask_sb,
                            )
                    return a_grp

                def emit_av_group(lj, gi, a_grp, first, last):
                    diag = gi == lj
                    ot = ot_banks[lj]
                    for c in range(4):
                        si = 4 * gi + c
                        off = 128 * c if diag else 0
                        nc.tensor.matmul(
                            ot[:, off:512],
                            v_sb[:, si, :],
                            a_grp[:, 512 * c + off : 512 * (c + 1)],
                            start=(first and c == 0),
                            stop=(last and c == 3),
                        )

                def epilogue(lj):
                    ot = ot_banks.pop(lj)
                    cl = slice(512 * lj, 512 * (lj + 1))
                    nc.vector.tensor_copy(out_sb[:, cl], ot[0 : E + 1, :])
                    nc.sync.dma_start(out=oo[pair][:, cl], in_=out_sb[:, cl])

                # ST order: bank-major, diagonal group first within each bank
                # (so its gpsimd mask-mult hides behind the bank's other
                # groups). AV order: diagonal group LAST (start flag on the
                # first-issued AV of each bank, stop on the diagonal).
                st_order = []
                av_order = []
                for lj in range(NB):
                    st_order.append((lj, lj))
                    st_order.extend((lj, gi) for gi in range(lj))
                    av_order.extend((lj, gi) for gi in range(lj))
                    av_order.append((lj, lj))
                a_cache = {}
                av_k = 0
                for st_k, (lj, gi) in enumerate(st_order):
                    if lj not in ot_banks:
                        ot_banks[lj] = otp.tile(
                            [128, 512], f32, tag="ot", name="ot"
                        )
                    a_cache[(lj, gi)] = emit_st_group(lj, gi)
                    # emit AV for any group whose ST was emitted >=2 slots
                    # ago, so its exp (which needs ~1.2us after the ST) can
                    # never block the PE queue head
                    while av_k < len(av_order) and av_order[av_k] in a_cache and \
                            st_order.index(av_order[av_k]) <= st_k - 2:
                        alj, agi = av_order[av_k]
                        emit_av_group(
                            alj, agi, a_cache.pop((alj, agi)),
                            first=(agi == 0 if alj > 0 else True),
                            last=(agi == alj),
                        )
                        if agi == alj:
                            epilogue(alj)
                        av_k += 1
                while av_k < len(av_order):
                    alj, agi = av_order[av_k]
                    emit_av_group(
                        alj, agi, a_cache.pop((alj, agi)),
                        first=(agi == 0 if alj > 0 else True),
                        last=(agi == alj),
                    )
                    if agi == alj:
                        epilogue(alj)
                    av_k += 1

    return nc


# ---------------------------------------------------------------------------
# Host-side sharding / unsharding
# ---------------------------------------------------------------------------

def _in_maps(queries, keys, values, tau, delta, st_dtype=bf16):
    np_st = mybir.dt.np(st_dtype)
    np_bf = mybir.dt.np(bf16)
    mask = np.triu(np.ones((128, 128), dtype=np.float32)).astype(np_bf)
    maps = []
    for c in range(N_CORES):
        ps = [2 * c, 2 * c + 1]
        b = ps[0] // H
        hs = [p % H for p in ps]
        qscale = np.float32(SCALE * tau[b, 0])
        expd = np.exp(SCALE * delta[b]).astype(np.float32)  # [L]
        qt = np.empty((PAIRS_PER_CORE, 128, L), dtype=np_st)
        kt = np.empty((PAIRS_PER_CORE, 128, L // 2), dtype=np_st)
        vv = np.zeros((PAIRS_PER_CORE, 128, NT, VW), dtype=np_bf)
        for i, h in enumerate(hs):
            qT = (queries[b, :, h, :].T * qscale).astype(np_st)  # [E, L]
            qt[i, 0:E] = qT
            qt[i, E:2 * E] = qT
            kT = keys[b, :, h, :].T.astype(np_st)  # [E, L]
            ktile = kT.reshape(E, NT, 128)
            kt[i, 0:E] = np.ascontiguousarray(
                ktile[:, 0::2, :]).reshape(E, L // 2)
            kt[i, E:2 * E] = np.ascontiguousarray(
                ktile[:, 1::2, :]).reshape(E, L // 2)
            vaug = np.zeros((L, VW), dtype=np.float32)
            vaug[:, 0:E] = values[b, :, h, :] * expd[:, None]
            vaug[:, E] = expd
            vv[i] = vaug.reshape(NT, 128, VW).transpose(1, 0, 2).astype(np_bf)
        maps.append({"qt": qt, "kt": kt, "vv": vv, "mask": mask})
    return maps


_CACHED = {}


def run(queries, keys, values, tau, delta, trace=False, st_dtype=bf16,
        av_dtype=None, dve_num=5, dve_den=8, mask_engine="gpsimd"):
    key = (str(st_dtype), dve_num, dve_den, mask_engine)
    if key not in _CACHED:
        _CACHED[key] = build_program(
            dve_num=dve_num, dve_den=dve_den, mask_engine=mask_engine,
            st_dtype=st_dtype)
    nc = _CACHED[key]
    in_maps = _in_maps(
        np.asarray(queries),
        np.asarray(keys),
        np.asarray(values),
        np.asarray(tau),
        np.asarray(delta),
        st_dtype=st_dtype,
    )
    res = run_bass_kernel_spmd(
        nc, in_maps, core_ids=list(range(N_CORES)), trace=trace
    )
    out = np.empty((B, L, H, E), dtype=np.float32)
    for c in range(N_CORES):
        o = res.results[c]["oo"]  # [2, E+1, L]
        for i, p in enumerate([2 * c, 2 * c + 1]):
            out[p // H, :, p % H, :] = (o[i, 0:E, :] / o[i, E, :]).T
    return out, res


def kernel(queries, keys, values, tau, delta):
    out, _ = run(queries, keys, values, tau, delta, trace=False)
    return out
